# revision 74
# baseline (speedup 1.0000x reference)
"""ContraFace loss kernel for 8 TRN2 NeuronCores.

Strategy: row-shard the [B, B] cosine matrix across 8 cores (1024 rows per
core). The device computes the raw (unmasked) row statistics of the scaled
cosine block: per-row sum of exp(S*cos) and per-row max of exp(S*cos); the
host removes the same-label / diagonal terms exactly (they are O(B) many)
and assembles the margin EMA + cross-entropy in float64.

Device pipeline per core, per (g, m) [128, 2048] tile of the [1024, 8192]
block:
  - PE: fp8(e4m3) DoubleRow matmuls, K=256 per instruction (2 per 512-wide
    quarter), accumulate a [128, 2048] PSUM tile in fp32. Host pre-scales
    operands (16*f1, 64*f2n) so e4m3 quantization error is benign; the
    random quantization noise averages out in the 8192-term row sums and
    the 8192-row margin mean (validated: rel err ~3e-4 end to end).
  - ACT: Exp with per-partition scale rn1/16 reads the PSUM tile directly
    and emits bf16 exp values to SBUF. ACT is the bottleneck engine
    (~61 us busy), so the schedule exists to keep it saturated: a dummy
    activation at t=0 preloads the Exp table during the DMA prologue, the
    first tile's Exp runs as 4x512 pieces chasing the first column-group's
    DMA quarters, and the last tile runs as 2x1024 so the final
    DVE drain chain is short.
  - DVE: two 4x-mode bf16 tensor_scalar passes over the exp values produce
    the row sum (op1=add accum) and row max (op1=max accum).
No mask tensor is loaded at all: same-label entries are included in the
raw stats and subtracted on the host (avg 2 per row), which removes 8 MB
of DMA traffic and the mask-multiply DVE pass.
"""

import sys

sys.path.insert(0, "/opt/trn_rl_repo")

import numpy as np
from contextlib import ExitStack

import ml_dtypes

from concourse import bass, bacc, tile
from concourse.bass_utils import run_bass_kernel_spmd
import concourse.mybir as mybir

dt = mybir.dt
Alu = mybir.AluOpType
Act = mybir.ActivationFunctionType
PM = mybir.MatmulPerfMode

B, D = 8192, 512
NCORES = 8
BS = B // NCORES          # 1024 rows per core
MT = BS // 128            # 8 M-tiles per core
G = 2048                  # column group width (4 PSUM banks as fp32)
NG = B // G               # 4 column groups
S = 64.0
EMA = 0.99
F1S = 16.0                # host pre-scale of f1 before e4m3 quantization
F2S = 64.0                # host pre-scale of f2n before e4m3 quantization
NSA = 76                  # statA cols: 32 max, 32 sum, (3+3)x2 ramp pieces
NSB = 4                   # statB cols: last-tile halves (2 max, 2 sum)

_prog_cache = {}


def _build_program():
    nc = bacc.Bacc(None)

    # fp0: the fused ramp block — f1 m-tiles 0/1 (cols 0:256) + f2 group-0
    # cols 0:512 (cols 256:768) in ONE tensor, so a single DMA (with a single
    # completion semaphore) unblocks the first matmuls as early as possible.
    fp0_d = nc.declare_dram_parameter("fp0", [128, 2, 2, 768], dt.float8e4, isOutput=False)
    g0p_d = [
        nc.declare_dram_parameter(f"g0p{p}", [128, 2, 2, 512], dt.float8e4, isOutput=False)
        for p in range(1, 4)
    ]
    f1m2_d = nc.declare_dram_parameter("f1m2", [128, 2, 2, 128], dt.float8e4, isOutput=False)
    f1r_d = nc.declare_dram_parameter("f1r", [128, 2, 2, BS - 384], dt.float8e4, isOutput=False)
    f2q_d = nc.declare_dram_parameter("f2q", [128, 2, 2, B], dt.float8e4, isOutput=False)
    srn1_d = nc.declare_dram_parameter("srn1", [128, MT], dt.float32, isOutput=False)
    smA_d = nc.declare_dram_parameter("smA", [128, NSA], dt.float32, isOutput=True)
    smB_d = nc.declare_dram_parameter("smB", [128, NSB], dt.float32, isOutput=True)

    with tile.TileContext(nc) as tc, ExitStack() as ctx:
        cst = ctx.enter_context(tc.tile_pool(name="cst", bufs=1))
        pan = ctx.enter_context(tc.tile_pool(name="pan", bufs=NG))
        exq = ctx.enter_context(tc.tile_pool(name="exq", bufs=6))
        dmp = ctx.enter_context(tc.tile_pool(name="dmp", bufs=2))
        psm = ctx.enter_context(
            tc.tile_pool(name="psm", bufs=2, space=bass.MemorySpace.PSUM)
        )

        statA = cst.tile([128, NSA], dt.float32, tag="statA")
        statB = cst.tile([128, NSB], dt.float32, tag="statB")
        srn1 = cst.tile([128, MT], dt.float32, tag="srn1")
        scr = cst.tile([128, 1], dt.bfloat16, tag="scr")
        fp0 = cst.tile([128, 2, 2, 768], dt.float8e4, tag="fp0")
        g0p = [cst.tile([128, 2, 2, 512], dt.float8e4, tag=f"g0p{p}", name=f"g0p{p}")
               for p in range(1, 4)]
        f1m2 = cst.tile([128, 2, 2, 128], dt.float8e4, tag="f1m2")
        f1r = cst.tile([128, 2, 2, BS - 384], dt.float8e4, tag="f1r")

        f2t = [None] + [pan.tile([128, 2, 2, G], dt.float8e4, tag="f2t", name=f"f2t{g}")
                        for g in range(1, NG)]

        # Preload the Exp activation table while DMAs run (1283 ns off the
        # ACT critical path). Output unused.
        one = nc.const_aps.aps[(dt.float32, 1.0)]
        nc.scalar.activation(scr[:], one, Act.Exp, bias=0.0, scale=1.0)
        # cols 31/63 (last tile) live in statB; zero statA so the writeback
        # reads fully-initialized memory
        nc.gpsimd.memset(statA[:], 0.0)

        # DMA order = need order, all data transfers on ONE queue so the
        # shared DMA device serves them strictly in need order (a second
        # queue's transfers would interleave and delay the ramp).
        nc.gpsimd.dma_start(srn1[:], srn1_d[:])
        nc.sync.dma_start(fp0[:], fp0_d[:])
        for p in range(3):
            nc.sync.dma_start(g0p[p][:], g0p_d[p][:])
        # m2's f1 slice arrives AFTER the bulk: in the scheduler's own sim
        # this pushes m2's matmuls past m1's Exp start, so the enforced
        # cross-engine alignment for m1's Exp does not wait on them
        nc.sync.dma_start(f1r[:], f1r_d[:])
        nc.sync.dma_start(f1m2[:], f1m2_d[:])
        # scheduling hint: the scheduler's sim runs DMAs ~1us faster than
        # the final timing (no sem-prop modeling in its readiness), so
        # cross-engine alignment waits it emits for the ramp Exps would
        # otherwise include these group transfers and stall ACT on them.
        # There is huge real slack (group g is consumed ~8*g us later), so
        # push them past the ramp in the scheduler's view.
        for g in range(1, NG):
            with tc.tile_wait_until(0.004 * g):
                nc.sync.dma_start(f2t[g][:], f2q_d[:][:, :, :, g * G:(g + 1) * G])

        def f1ap(m, c):
            if m < 2:
                return fp0[:, c, :, m * 128:(m + 1) * 128]
            if m == 2:
                return f1m2[:, c, :, :]
            return f1r[:, c, :, (m - 3) * 128:(m - 2) * 128]

        def mm_cols(acc, g, m, c0, c1, acc0=0):
            for c in range(2):
                nc.tensor.matmul(
                    acc[:, c0 - acc0:c1 - acc0],
                    f1ap(m, c),
                    f2t[g][:, c, :, c0:c1],
                    start=(c == 0),
                    stop=(c == 1),
                    perf_mode=PM.DoubleRow,
                )

        def mm_piece(acc, m, p, acc0=0, split=False):
            # group-0 quarter p: quarter 0 lives in fp0 cols 256:768
            src = fp0[:, :, :, 256:768] if p == 0 else g0p[p - 1][:]
            # split=True emits 2x256-wide matmuls per contraction chunk (4
            # per piece): engine-sem increments are batched every 8 PE
            # instructions, and 16 ramp matmuls put the batch boundary
            # exactly at m1's last matmul instead of the middle of m2
            for h in range(2 if split else 1):
                for c in range(2):
                    w = 256 if split else 512
                    o = p * 512 + h * 256
                    nc.tensor.matmul(
                        acc[:, o - acc0:o + w - acc0],
                        f1ap(m, c),
                        src[:, c, :, h * 256:h * 256 + w],
                        start=(c == 0),
                        stop=(c == 1),
                        perf_mode=PM.DoubleRow,
                    )

        def act_dve(src_ap, m, width, stat, mxc, sec, name):
            ex = exq.tile([128, width], dt.bfloat16, tag="ex", name=f"ex{name}")
            nc.scalar.activation(
                ex[:], src_ap, Act.Exp, bias=0.0, scale=srn1[:, m:m + 1]
            )
            dums = dmp.tile([128, width], dt.bfloat16, tag="dums", name=f"ds{name}")
            nc.vector.tensor_scalar(
                out=dums[:], in0=ex[:], scalar1=1.0, scalar2=None,
                op0=Alu.mult, op1=Alu.add, accum_out=stat[:, sec:sec + 1],
            )
            dumm = dmp.tile([128, width], dt.bfloat16, tag="dumm", name=f"dm{name}")
            nc.vector.tensor_scalar(
                out=dumm[:], in0=ex[:], scalar1=1.0, scalar2=None,
                op0=Alu.mult, op1=Alu.max, accum_out=stat[:, mxc:mxc + 1],
            )

        for g in range(NG):
            for m in range(MT):
                col = g * MT + m
                if g == 0 and m == 0:
                    # ramp: m0's Exp runs piecewise chasing the group-0 DMA
                    # stream; m1's matmuls interleave per piece so its full
                    # 2048-wide Exp is ready the moment the pieces finish.
                    # Ramp order [m0p0, m0p1, m0p2, m1a, m0p3, m1b]: each
                    # piece in its OWN psum tile (no tile-granular WAR can
                    # serialize the ramp), and m1's first half — whose f2
                    # columns arrived with the earliest DMAs — slots in
                    # BEFORE m0's last piece so ACT chews ready work while
                    # the final group-0 quarter is still in flight.
                    def m0_piece(p):
                        accp = psm.tile([128, 512], dt.float32, tag="acc",
                                        name=f"accp{p}")
                        mm_piece(accp, 0, p, acc0=p * 512, split=True)
                        mxc = 0 if p == 0 else 63 + p
                        sec = 32 if p == 0 else 66 + p
                        act_dve(accp[:], 0, 512, statA, mxc, sec, f"p{p}")

                    # tile_wait_until hints: the scheduler's sim does not
                    # model semaphore-batch latencies, so it snapshots
                    # engine clocks ~0.4-0.9us ahead of reality at each
                    # alignment point; pinning the ramp steps near their
                    # real times keeps those snapshots honest.
                    for p in range(3):
                        with tc.tile_wait_until(0.0040 + 0.0006 * p):
                            m0_piece(p)
                    acc1a = psm.tile([128, 1024], dt.float32, tag="acc", name="acc1a")
                    mm_piece(acc1a, 1, 1)
                    mm_piece(acc1a, 1, 0)
                    with tc.tile_wait_until(0.0060):
                        act_dve(acc1a[:], 1, 1024, statA, 1, 33, "g0m1a")
                    with tc.tile_wait_until(0.0066):
                        m0_piece(3)
                    acc1b = psm.tile([128, 1024], dt.float32, tag="acc", name="acc1b")
                    mm_piece(acc1b, 1, 3, acc0=1024)
                    mm_piece(acc1b, 1, 2, acc0=1024)
                    with tc.tile_wait_until(0.0074):
                        act_dve(acc1b[:], 1, 1024, statA, 70, 71, "g0m1b")
                elif g == 0 and m == 1:
                    continue
                elif g == 0:
                    acc = psm.tile([128, G], dt.float32, tag="acc", name="acc")
                    for p in range(4):
                        mm_piece(acc, m, p)
                    act_dve(acc[:], m, G, statA, col, 32 + col, f"g{g}m{m}")
                elif g == NG - 1 and m == MT - 1:
                    # early writeback of everything except the last tile
                    nc.gpsimd.dma_start(smA_d[:], statA[:])
                    # tail: 1024-wide halves in SEPARATE psum tiles so h1's
                    # matmuls don't hit a tile-granular WAR wait on h0's read
                    for h in range(2):
                        acc = psm.tile([128, G // 2], dt.float32, tag="acc",
                                       name=f"acct{h}")
                        mm_cols(acc, g, m, h * 1024, h * 1024 + 512, h * 1024)
                        mm_cols(acc, g, m, h * 1024 + 512, (h + 1) * 1024, h * 1024)
                        act_dve(acc[:], m, 1024, statB, h, 2 + h, f"t{h}")
                else:
                    acc = psm.tile([128, G], dt.float32, tag="acc", name="acc")
                    for q in range(4):
                        mm_cols(acc, g, m, q * 512, (q + 1) * 512)
                    act_dve(acc[:], m, G, statA, col, 32 + col, f"g{g}m{m}")
        nc.sync.dma_start(smB_d[:], statB[:])

    if not nc.is_finalized():
        nc.finalize()
    return nc


def _get_program():
    if "nc" not in _prog_cache:
        _prog_cache["nc"] = _build_program()
    return _prog_cache["nc"]


def _quant_layout(mat_t):
    """[D, N] fp8 matrix -> [128, 2, 2, N] with d = c*256 + i*128 + p."""
    N = mat_t.shape[1]
    return np.ascontiguousarray(
        mat_t.reshape(2, 2, 128, N).transpose(2, 0, 1, 3)
    )


def _host_inputs(feature1, feature2, label):
    e4 = ml_dtypes.float8_e4m3fn
    f1 = np.asarray(feature1, dtype=np.float32)
    f2 = np.asarray(feature2, dtype=np.float32)

    f2n64 = f2.astype(np.float64)
    f2n = (f2n64 / np.linalg.norm(f2n64, axis=1, keepdims=True)).astype(np.float32)
    Bq = (F2S * f2n).astype(e4)                       # [B, D]
    f2q = _quant_layout(np.ascontiguousarray(Bq.T))   # [128, 2, 2, B]

    rn1 = 1.0 / np.linalg.norm(f1.astype(np.float64), axis=1)

    g0ps = {
        f"g0p{p}": np.ascontiguousarray(f2q[:, :, :, p * 512:(p + 1) * 512])
        for p in range(1, 4)
    }

    in_maps = []
    for c in range(NCORES):
        sl = slice(c * BS, (c + 1) * BS)
        Aq = (F1S * f1[sl]).astype(e4)                # [BS, D]
        f1q = _quant_layout(np.ascontiguousarray(Aq.T))
        srn1 = np.ascontiguousarray(
            (rn1[sl] * (S / (F1S * F2S))).reshape(MT, 128).T.astype(np.float32)
        )
        fp0 = np.concatenate([f1q[:, :, :, 0:256], f2q[:, :, :, 0:512]], axis=3)
        in_maps.append(dict(
            fp0=np.ascontiguousarray(fp0),
            f1m2=np.ascontiguousarray(f1q[:, :, :, 256:384]),
            f1r=np.ascontiguousarray(f1q[:, :, :, 384:BS]),
            f2q=f2q, srn1=srn1, **g0ps,
        ))
    return in_maps


def kernel(feature1, feature2, label, _want_results=False, _trace=False):
    e4 = ml_dtypes.float8_e4m3fn
    f1 = np.asarray(feature1, dtype=np.float32)
    f2 = np.asarray(feature2, dtype=np.float32)
    lab = np.asarray(label)
    in_maps = _host_inputs(f1, f2, lab)

    nc = _get_program()
    kw = dict(trace=True) if _trace else {}
    out = run_bass_kernel_spmd(nc, in_maps, list(range(NCORES)), **kw)
    res = out.results

    # ---- host O(B) combine in float64 ----
    f1_64 = f1.astype(np.float64)
    f2_64 = f2.astype(np.float64)
    rn1 = 1.0 / np.linalg.norm(f1_64, axis=1)
    rn2 = 1.0 / np.linalg.norm(f2_64, axis=1)
    pos = np.clip(np.einsum("ij,ij->i", f1_64, f2_64) * rn1 * rn2, -1.0, 1.0)

    # gather device stats: row r = c*BS + m*128 + p, statA col = g*MT + m
    MX = np.empty(B, dtype=np.float64)
    SE = np.empty(B, dtype=np.float64)
    for c in range(NCORES):
        smA = res[c]["smA"].astype(np.float64)        # [128, 70]
        smB = res[c]["smB"].astype(np.float64)        # [128, 4]
        mxs = smA[:, :NG * MT].copy()                 # [128, 32]
        ses = smA[:, NG * MT:2 * NG * MT].copy()
        # fold ramp pieces: m0 extras (max 64..66, sum 67..69) and m1's
        # second half (max 70, sum 71)
        mxs[:, 0] = np.maximum(mxs[:, 0], smA[:, 64:67].max(axis=1))
        ses[:, 0] += smA[:, 67:70].sum(axis=1)
        mxs[:, 1] = np.maximum(mxs[:, 1], smA[:, 70])
        ses[:, 1] += smA[:, 71]
        # fold last-tile halves from smB into (g3, m7) = col 31
        mxs[:, NG * MT - 1] = np.maximum(smB[:, 0], smB[:, 1])
        ses[:, NG * MT - 1] = smB[:, 2] + smB[:, 3]
        mx = mxs.reshape(128, NG, MT).max(axis=1)     # [128, MT]
        se = ses.reshape(128, NG, MT).sum(axis=1)
        sl = slice(c * BS, (c + 1) * BS)
        MX[sl] = mx.T.reshape(BS)
        SE[sl] = se.T.reshape(BS)

    # same-label corrections with the device's quantized operands
    f2n = (f2_64 / np.linalg.norm(f2_64, axis=1, keepdims=True)).astype(np.float32)
    Aq_all = (F1S * f1).astype(e4).astype(np.float32)
    Bq_all = (F2S * f2n).astype(e4).astype(np.float32)
    scale = rn1 * (S / (F1S * F2S))

    order = np.argsort(lab, kind="stable")
    labs = lab[order]
    starts = np.r_[0, np.flatnonzero(np.diff(labs)) + 1, len(labs)]
    corr = np.zeros(B)
    cnt = np.zeros(B)
    for a, b in zip(starts[:-1], starts[1:]):
        idx = order[a:b]
        blk = (Aq_all[idx] @ Bq_all[idx].T) * scale[idx][:, None]
        corr[idx] = np.exp(blk.astype(np.float64)).sum(axis=1)
        cnt[idx] = len(idx)

    neg = np.log(MX) / S
    m = EMA * np.mean(pos - neg)
    z = S * (pos - m)
    sumoff = SE - corr + (cnt - 1.0)
    loss = np.mean(np.log(sumoff + np.exp(z)) - z)
    out_val = np.float32(loss)
    if _want_results:
        return out_val, out
    return out_val


# revision 76
# speedup vs baseline: 1.0248x; 1.0248x over previous
"""ContraFace loss kernel for 8 TRN2 NeuronCores.

Strategy: row-shard the [B, B] cosine matrix across 8 cores (1024 rows per
core). The device computes the raw (unmasked) row statistics of the scaled
cosine block: per-row sum of exp(S*cos) and per-row max of exp(S*cos); the
host removes the same-label / diagonal terms exactly (they are O(B) many)
and assembles the margin EMA + cross-entropy in float64.

Device pipeline per core, per (g, m) [128, 2048] tile of the [1024, 8192]
block:
  - PE: fp8(e4m3) DoubleRow matmuls, K=256 per instruction (2 per 512-wide
    quarter), accumulate a [128, 2048] PSUM tile in fp32. Host pre-scales
    operands (16*f1, 64*f2n) so e4m3 quantization error is benign; the
    random quantization noise averages out in the 8192-term row sums and
    the 8192-row margin mean (validated: rel err ~3e-4 end to end).
  - ACT: Exp with per-partition scale rn1/16 reads the PSUM tile directly
    and emits bf16 exp values to SBUF. ACT is the bottleneck engine
    (~61 us busy), so the schedule exists to keep it saturated: a dummy
    activation at t=0 preloads the Exp table during the DMA prologue, the
    first tile's Exp runs as 4x512 pieces chasing the first column-group's
    DMA quarters, and the last tile runs as 2x1024 so the final
    DVE drain chain is short.
  - DVE: two 4x-mode bf16 tensor_scalar passes over the exp values produce
    the row sum (op1=add accum) and row max (op1=max accum).
No mask tensor is loaded at all: same-label entries are included in the
raw stats and subtracted on the host (avg 2 per row), which removes 8 MB
of DMA traffic and the mask-multiply DVE pass.
"""

import sys

sys.path.insert(0, "/opt/trn_rl_repo")

import numpy as np
from contextlib import ExitStack

import ml_dtypes

from concourse import bass, bacc, tile
from concourse.bass_utils import run_bass_kernel_spmd
import concourse.mybir as mybir

dt = mybir.dt
Alu = mybir.AluOpType
Act = mybir.ActivationFunctionType
PM = mybir.MatmulPerfMode

B, D = 8192, 512
NCORES = 8
BS = B // NCORES          # 1024 rows per core
MT = BS // 128            # 8 M-tiles per core
G = 2048                  # column group width (4 PSUM banks as fp32)
NG = B // G               # 4 column groups
S = 64.0
EMA = 0.99
F1S = 16.0                # host pre-scale of f1 before e4m3 quantization
F2S = 64.0                # host pre-scale of f2n before e4m3 quantization
NSA = 76                  # statA cols: 32 max, 32 sum, (3+3)x2 ramp pieces
NSB = 4                   # statB cols: last-tile halves (2 max, 2 sum)

_prog_cache = {}


def _build_program():
    nc = bacc.Bacc(None)

    # fp0: the fused ramp block — f1 m-tiles 0/1 (cols 0:256) + f2 group-0
    # cols 0:512 (cols 256:768) in ONE tensor, so a single DMA (with a single
    # completion semaphore) unblocks the first matmuls as early as possible.
    fp0_d = nc.declare_dram_parameter("fp0", [128, 2, 2, 768], dt.float8e4, isOutput=False)
    g0p_d = [
        nc.declare_dram_parameter(f"g0p{p}", [128, 2, 2, 512], dt.float8e4, isOutput=False)
        for p in range(1, 4)
    ]
    f1m2_d = nc.declare_dram_parameter("f1m2", [128, 2, 2, 128], dt.float8e4, isOutput=False)
    f1r_d = nc.declare_dram_parameter("f1r", [128, 2, 2, BS - 384], dt.float8e4, isOutput=False)
    f2q_d = nc.declare_dram_parameter("f2q", [128, 2, 2, B], dt.float8e4, isOutput=False)
    srn1_d = nc.declare_dram_parameter("srn1", [128, MT], dt.float32, isOutput=False)
    smA_d = nc.declare_dram_parameter("smA", [128, NSA], dt.float32, isOutput=True)
    smB_d = nc.declare_dram_parameter("smB", [128, NSB], dt.float32, isOutput=True)

    with tile.TileContext(nc) as tc, ExitStack() as ctx:
        cst = ctx.enter_context(tc.tile_pool(name="cst", bufs=1))
        pan = ctx.enter_context(tc.tile_pool(name="pan", bufs=NG))
        exq = ctx.enter_context(tc.tile_pool(name="exq", bufs=6))
        dmp = ctx.enter_context(tc.tile_pool(name="dmp", bufs=4))
        psm = ctx.enter_context(
            tc.tile_pool(name="psm", bufs=2, space=bass.MemorySpace.PSUM)
        )

        statA = cst.tile([128, NSA], dt.float32, tag="statA")
        statB = cst.tile([128, NSB], dt.float32, tag="statB")
        srn1 = cst.tile([128, MT], dt.float32, tag="srn1")
        scr = cst.tile([128, 1], dt.bfloat16, tag="scr")
        fp0 = cst.tile([128, 2, 2, 768], dt.float8e4, tag="fp0")
        g0p = [cst.tile([128, 2, 2, 512], dt.float8e4, tag=f"g0p{p}", name=f"g0p{p}")
               for p in range(1, 4)]
        f1m2 = cst.tile([128, 2, 2, 128], dt.float8e4, tag="f1m2")
        f1r = cst.tile([128, 2, 2, BS - 384], dt.float8e4, tag="f1r")

        f2t = [None] + [pan.tile([128, 2, 2, G], dt.float8e4, tag="f2t", name=f"f2t{g}")
                        for g in range(1, NG)]

        # Preload the Exp activation table while DMAs run (1283 ns off the
        # ACT critical path). Output unused.
        one = nc.const_aps.aps[(dt.float32, 1.0)]
        nc.scalar.activation(scr[:], one, Act.Exp, bias=0.0, scale=1.0)
        # cols 31/63 (last tile) live in statB; zero statA so the writeback
        # reads fully-initialized memory
        nc.gpsimd.memset(statA[:], 0.0)

        # DMA order = need order, all data transfers on ONE queue so the
        # shared DMA device serves them strictly in need order (a second
        # queue's transfers would interleave and delay the ramp).
        nc.gpsimd.dma_start(srn1[:], srn1_d[:])
        nc.sync.dma_start(fp0[:], fp0_d[:])
        for p in range(3):
            nc.sync.dma_start(g0p[p][:], g0p_d[p][:])
        # m2's f1 slice arrives AFTER the bulk: in the scheduler's own sim
        # this pushes m2's matmuls past m1's Exp start, so the enforced
        # cross-engine alignment for m1's Exp does not wait on them
        nc.sync.dma_start(f1r[:], f1r_d[:])
        nc.sync.dma_start(f1m2[:], f1m2_d[:])
        # scheduling hint: the scheduler's sim runs DMAs ~1us faster than
        # the final timing (no sem-prop modeling in its readiness), so
        # cross-engine alignment waits it emits for the ramp Exps would
        # otherwise include these group transfers and stall ACT on them.
        # There is huge real slack (group g is consumed ~8*g us later), so
        # push them past the ramp in the scheduler's view.
        for g in range(1, NG):
            with tc.tile_wait_until(0.004 * g):
                nc.sync.dma_start(f2t[g][:], f2q_d[:][:, :, :, g * G:(g + 1) * G])

        def f1ap(m, c):
            if m < 2:
                return fp0[:, c, :, m * 128:(m + 1) * 128]
            if m == 2:
                return f1m2[:, c, :, :]
            return f1r[:, c, :, (m - 3) * 128:(m - 2) * 128]

        def mm_cols(acc, g, m, c0, c1, acc0=0):
            for c in range(2):
                nc.tensor.matmul(
                    acc[:, c0 - acc0:c1 - acc0],
                    f1ap(m, c),
                    f2t[g][:, c, :, c0:c1],
                    start=(c == 0),
                    stop=(c == 1),
                    perf_mode=PM.DoubleRow,
                )

        def mm_piece(acc, m, p, acc0=0, split=False):
            # group-0 quarter p: quarter 0 lives in fp0 cols 256:768
            src = fp0[:, :, :, 256:768] if p == 0 else g0p[p - 1][:]
            # split=True emits 2x256-wide matmuls per contraction chunk (4
            # per piece): engine-sem increments are batched every 8 PE
            # instructions, and 16 ramp matmuls put the batch boundary
            # exactly at m1's last matmul instead of the middle of m2
            for h in range(2 if split else 1):
                for c in range(2):
                    w = 256 if split else 512
                    o = p * 512 + h * 256
                    nc.tensor.matmul(
                        acc[:, o - acc0:o + w - acc0],
                        f1ap(m, c),
                        src[:, c, :, h * 256:h * 256 + w],
                        start=(c == 0),
                        stop=(c == 1),
                        perf_mode=PM.DoubleRow,
                    )

        def act_dve(src_ap, m, width, stat, mxc, sec, name):
            ex = exq.tile([128, width], dt.bfloat16, tag="ex", name=f"ex{name}")
            nc.scalar.activation(
                ex[:], src_ap, Act.Exp, bias=0.0, scale=srn1[:, m:m + 1]
            )
            dums = dmp.tile([128, width], dt.bfloat16, tag="dums", name=f"ds{name}")
            nc.vector.tensor_scalar(
                out=dums[:], in0=ex[:], scalar1=1.0, scalar2=None,
                op0=Alu.mult, op1=Alu.add, accum_out=stat[:, sec:sec + 1],
            )
            dumm = dmp.tile([128, width], dt.bfloat16, tag="dumm", name=f"dm{name}")
            nc.vector.tensor_scalar(
                out=dumm[:], in0=ex[:], scalar1=1.0, scalar2=None,
                op0=Alu.mult, op1=Alu.max, accum_out=stat[:, mxc:mxc + 1],
            )

        for g in range(NG):
            for m in range(MT):
                col = g * MT + m
                if g == 0 and m == 0:
                    # ramp: m0's Exp runs piecewise chasing the group-0 DMA
                    # stream; m1's matmuls interleave per piece so its full
                    # 2048-wide Exp is ready the moment the pieces finish.
                    # Ramp order [m0p0, m0p1, m0p2, m1a, m0p3, m1b]: each
                    # piece in its OWN psum tile (no tile-granular WAR can
                    # serialize the ramp), and m1's first half — whose f2
                    # columns arrived with the earliest DMAs — slots in
                    # BEFORE m0's last piece so ACT chews ready work while
                    # the final group-0 quarter is still in flight.
                    def m0_piece(p):
                        accp = psm.tile([128, 512], dt.float32, tag="acc",
                                        name=f"accp{p}")
                        mm_piece(accp, 0, p, acc0=p * 512, split=True)
                        mxc = 0 if p == 0 else 63 + p
                        sec = 32 if p == 0 else 66 + p
                        act_dve(accp[:], 0, 512, statA, mxc, sec, f"p{p}")

                    for p in range(3):
                        m0_piece(p)
                    acc1a = psm.tile([128, 1024], dt.float32, tag="acc", name="acc1a")
                    mm_piece(acc1a, 1, 1)
                    mm_piece(acc1a, 1, 0)
                    act_dve(acc1a[:], 1, 1024, statA, 1, 33, "g0m1a")
                    m0_piece(3)
                    acc1b = psm.tile([128, 1024], dt.float32, tag="acc", name="acc1b")
                    mm_piece(acc1b, 1, 3, acc0=1024)
                    mm_piece(acc1b, 1, 2, acc0=1024)
                    act_dve(acc1b[:], 1, 1024, statA, 70, 71, "g0m1b")
                elif g == 0 and m == 1:
                    continue
                elif g == 0:
                    acc = psm.tile([128, G], dt.float32, tag="acc", name="acc")
                    for p in range(4):
                        mm_piece(acc, m, p)
                    act_dve(acc[:], m, G, statA, col, 32 + col, f"g{g}m{m}")
                elif g == NG - 1 and m == MT - 1:
                    # early writeback of everything except the last tile
                    nc.gpsimd.dma_start(smA_d[:], statA[:])
                    # tail: 1024-wide halves in SEPARATE psum tiles so h1's
                    # matmuls don't hit a tile-granular WAR wait on h0's read
                    for h in range(2):
                        acc = psm.tile([128, G // 2], dt.float32, tag="acc",
                                       name=f"acct{h}")
                        mm_cols(acc, g, m, h * 1024, h * 1024 + 512, h * 1024)
                        mm_cols(acc, g, m, h * 1024 + 512, (h + 1) * 1024, h * 1024)
                        act_dve(acc[:], m, 1024, statB, h, 2 + h, f"t{h}")
                else:
                    acc = psm.tile([128, G], dt.float32, tag="acc", name="acc")
                    for q in range(4):
                        mm_cols(acc, g, m, q * 512, (q + 1) * 512)
                    act_dve(acc[:], m, G, statA, col, 32 + col, f"g{g}m{m}")
        nc.sync.dma_start(smB_d[:], statB[:])

    if not nc.is_finalized():
        nc.finalize()
    return nc


def _get_program():
    if "nc" not in _prog_cache:
        _prog_cache["nc"] = _build_program()
    return _prog_cache["nc"]


def _quant_layout(mat_t):
    """[D, N] fp8 matrix -> [128, 2, 2, N] with d = c*256 + i*128 + p."""
    N = mat_t.shape[1]
    return np.ascontiguousarray(
        mat_t.reshape(2, 2, 128, N).transpose(2, 0, 1, 3)
    )


def _host_inputs(feature1, feature2, label):
    e4 = ml_dtypes.float8_e4m3fn
    f1 = np.asarray(feature1, dtype=np.float32)
    f2 = np.asarray(feature2, dtype=np.float32)

    f2n64 = f2.astype(np.float64)
    f2n = (f2n64 / np.linalg.norm(f2n64, axis=1, keepdims=True)).astype(np.float32)
    Bq = (F2S * f2n).astype(e4)                       # [B, D]
    f2q = _quant_layout(np.ascontiguousarray(Bq.T))   # [128, 2, 2, B]

    rn1 = 1.0 / np.linalg.norm(f1.astype(np.float64), axis=1)

    g0ps = {
        f"g0p{p}": np.ascontiguousarray(f2q[:, :, :, p * 512:(p + 1) * 512])
        for p in range(1, 4)
    }

    in_maps = []
    for c in range(NCORES):
        sl = slice(c * BS, (c + 1) * BS)
        Aq = (F1S * f1[sl]).astype(e4)                # [BS, D]
        f1q = _quant_layout(np.ascontiguousarray(Aq.T))
        srn1 = np.ascontiguousarray(
            (rn1[sl] * (S / (F1S * F2S))).reshape(MT, 128).T.astype(np.float32)
        )
        fp0 = np.concatenate([f1q[:, :, :, 0:256], f2q[:, :, :, 0:512]], axis=3)
        in_maps.append(dict(
            fp0=np.ascontiguousarray(fp0),
            f1m2=np.ascontiguousarray(f1q[:, :, :, 256:384]),
            f1r=np.ascontiguousarray(f1q[:, :, :, 384:BS]),
            f2q=f2q, srn1=srn1, **g0ps,
        ))
    return in_maps


def kernel(feature1, feature2, label, _want_results=False, _trace=False):
    e4 = ml_dtypes.float8_e4m3fn
    f1 = np.asarray(feature1, dtype=np.float32)
    f2 = np.asarray(feature2, dtype=np.float32)
    lab = np.asarray(label)
    in_maps = _host_inputs(f1, f2, lab)

    nc = _get_program()
    kw = dict(trace=True) if _trace else {}
    out = run_bass_kernel_spmd(nc, in_maps, list(range(NCORES)), **kw)
    res = out.results

    # ---- host O(B) combine in float64 ----
    f1_64 = f1.astype(np.float64)
    f2_64 = f2.astype(np.float64)
    rn1 = 1.0 / np.linalg.norm(f1_64, axis=1)
    rn2 = 1.0 / np.linalg.norm(f2_64, axis=1)
    pos = np.clip(np.einsum("ij,ij->i", f1_64, f2_64) * rn1 * rn2, -1.0, 1.0)

    # gather device stats: row r = c*BS + m*128 + p, statA col = g*MT + m
    MX = np.empty(B, dtype=np.float64)
    SE = np.empty(B, dtype=np.float64)
    for c in range(NCORES):
        smA = res[c]["smA"].astype(np.float64)        # [128, 70]
        smB = res[c]["smB"].astype(np.float64)        # [128, 4]
        mxs = smA[:, :NG * MT].copy()                 # [128, 32]
        ses = smA[:, NG * MT:2 * NG * MT].copy()
        # fold ramp pieces: m0 extras (max 64..66, sum 67..69) and m1's
        # second half (max 70, sum 71)
        mxs[:, 0] = np.maximum(mxs[:, 0], smA[:, 64:67].max(axis=1))
        ses[:, 0] += smA[:, 67:70].sum(axis=1)
        mxs[:, 1] = np.maximum(mxs[:, 1], smA[:, 70])
        ses[:, 1] += smA[:, 71]
        # fold last-tile halves from smB into (g3, m7) = col 31
        mxs[:, NG * MT - 1] = np.maximum(smB[:, 0], smB[:, 1])
        ses[:, NG * MT - 1] = smB[:, 2] + smB[:, 3]
        mx = mxs.reshape(128, NG, MT).max(axis=1)     # [128, MT]
        se = ses.reshape(128, NG, MT).sum(axis=1)
        sl = slice(c * BS, (c + 1) * BS)
        MX[sl] = mx.T.reshape(BS)
        SE[sl] = se.T.reshape(BS)

    # same-label corrections with the device's quantized operands
    f2n = (f2_64 / np.linalg.norm(f2_64, axis=1, keepdims=True)).astype(np.float32)
    Aq_all = (F1S * f1).astype(e4).astype(np.float32)
    Bq_all = (F2S * f2n).astype(e4).astype(np.float32)
    scale = rn1 * (S / (F1S * F2S))

    order = np.argsort(lab, kind="stable")
    labs = lab[order]
    starts = np.r_[0, np.flatnonzero(np.diff(labs)) + 1, len(labs)]
    corr = np.zeros(B)
    cnt = np.zeros(B)
    for a, b in zip(starts[:-1], starts[1:]):
        idx = order[a:b]
        blk = (Aq_all[idx] @ Bq_all[idx].T) * scale[idx][:, None]
        corr[idx] = np.exp(blk.astype(np.float64)).sum(axis=1)
        cnt[idx] = len(idx)

    neg = np.log(MX) / S
    m = EMA * np.mean(pos - neg)
    z = S * (pos - m)
    sumoff = SE - corr + (cnt - 1.0)
    loss = np.mean(np.log(sumoff + np.exp(z)) - z)
    out_val = np.float32(loss)
    if _want_results:
        return out_val, out
    return out_val
